# revision 46
# baseline (speedup 1.0000x reference)
"""Deformable Conv2d (offset-conv -> bilinear sample -> 3x3 conv) on 8 NeuronCores.

Sharding: batch(4) x H-halves(2) -> 8 cores. Each core computes a [64, 64, 128]
slice of the output for one image. Inputs per core: a zero-padded halo slice of
its image plus (replicated) weights and index-offset constants.

Per-core device pipeline (v2):
  1. xh (f16, with the one-column-shifted copy on partitions 64-127) is built
     on the HOST and DMA'd straight into SBUF (no device cast / shift).
  2. offset conv emitted in 4 row-groups, interleaved with the 4 chunk preps
     so chunk 0's gathers start ~15us in instead of after the whole conv.
  3. prep(u): PE-transpose offsets to pixel-partitioned layout, DVE index
     math -> gather indices (int16) + bilinear corner weight products; the
     weight products are stored c-PAIR-DUPLICATED (innermost dim [2] step-1)
     so the weighting multiply qualifies for the DVE 2x fp16 perf mode
     (step-0 broadcast only one level up). Gathers for all 9 taps emitted
     here too - the v-tile pool ring throttles Pool run-ahead.
  4. dma_gather (Pool/SWDGE): one 512B descriptor per (tap, output pixel)
     fetches all 4 bilinear corners for all 64 channels.
  5. weight the gathered corners on DVE (2x-rate mult + pair add)
  6. PE transposes to put (y-corner, channel) on partitions
  7. deform conv: PE matmuls contracting (y-corner, channel) per tap,
     accumulating the 9 taps in PSUM; bias via ACT on eviction.
"""

import numpy as np
from contextlib import ExitStack

B, C, H, W, O = 4, 64, 128, 128, 64
K2, CH = 9, 18
NI = 64               # output rows per core
HALO = 4
RH, RW = 72, 136      # halo slice dims (rows [h*64-4, h*64+68), cols [-4, 132))
NPIX = RH * RW        # 9792
TCH = 77              # ceil(NPIX/128) transpose chunks for the gather table
XHF = TCH * 128       # 9856 padded pixel count
TROWS = XHF           # gather-table rows (one per padded pixel)
Q = NI * W            # 8192 output pixels per core
ICH = 16              # i-rows per main-loop chunk
NCHUNK = NI // ICH    # 4 chunks
NIDX = ICH * W        # 2048 gather indices per (tap, chunk)
YCL = 70.99
XCL = 134.99

_cache = {}


def _ch_perm(ch):
    # offset-conv output channel order: ch in [0,9) -> oy of tap ch,
    # ch in [9,18) -> ox of tap ch-9. Source channel in w_off layout:
    return 2 * ch if ch < 9 else 2 * (ch - 9) + 1


def _build_consts(b_off):
    """Host-side constant tensors (identical for every core). The offset-conv
    bias is folded in here (cadd is added to the raw conv output). The y base
    is CHUNK-RELATIVE (i%16), so cadd is 16-periodic in i and only one chunk's
    worth ([128, ICH*CH]) is stored: every chunk adds the same tile."""
    cadd = np.zeros((128, ICH, CH), dtype=np.float32)
    for chn in range(CH):
        if chn < 9:
            kh = chn // 3
            cadd[:, :, chn] = (np.arange(ICH, dtype=np.float32) + 3 + kh)[None, :]
        else:
            kw = (chn - 9) % 3
            cadd[:, :, chn] = (np.arange(128, dtype=np.float32) + 3 + kw)[:, None]
        cadd[:, :, chn] += b_off[_ch_perm(chn)]
    return cadd.reshape(128, ICH * CH)


def _prep_weights(w_off, b_off, w_dcn, b_dcn):
    # Offset-conv lhsT, packed for double-tap contraction: for each kernel row
    # kh, taps (kh,0) and (kh,1) contract together over K=128 (the image copy
    # on partitions 64-127 is pre-shifted one column), tap (kh,2) is a K=64
    # single. woffp[kh]: [128, 18]; woffs[kh]: [64, 18].
    woffp = np.zeros((3, 2 * C, CH), dtype=np.float32)
    woffs = np.zeros((3, C, CH), dtype=np.float32)
    for kh in range(3):
        for chn in range(CH):
            woffp[kh, :C, chn] = w_off[_ch_perm(chn), :, kh, 0]
            woffp[kh, C:, chn] = w_off[_ch_perm(chn), :, kh, 1]
            woffs[kh, :, chn] = w_off[_ch_perm(chn), :, kh, 2]
    # wdcn_r[k, a*64+c, o] : lhsT for deform conv tap k, replicated over the
    # y-corner index a (the transposed sampled tensor has (a, c) on partitions)
    wdcn_r = np.zeros((K2, 2 * C, O), dtype=np.float32)
    for k in range(K2):
        kh, kw = k // 3, k % 3
        wdcn_r[k, :C, :] = w_dcn[:, :, kh, kw].T
        wdcn_r[k, C:, :] = w_dcn[:, :, kh, kw].T
    # device-layout f16 weights (partition-major, tap-interleaved free dim)
    woffph = np.ascontiguousarray(
        woffp.transpose(1, 0, 2).reshape(2 * C, 3 * CH)
    ).astype(np.float16)
    woffsh = np.ascontiguousarray(
        woffs.transpose(1, 0, 2).reshape(C, 3 * CH)
    ).astype(np.float16)
    wdcnh = np.ascontiguousarray(
        wdcn_r.transpose(1, 0, 2).reshape(2 * C, K2 * O)
    ).astype(np.float16)
    return woffph, woffsh, wdcnh, b_dcn.reshape(O, 1).astype(np.float32)


def build_tile_kernel(nc, ins, out_ap, stage=99, repeat=1):
    """Emit the per-core program. ins: dict name -> AP (DRAM).
    stage truncates the pipeline for debugging (99 = full)."""
    import concourse.bass as bass
    import concourse.mybir as mybir
    import concourse.tile as tile
    from concourse.masks import make_identity

    f32 = mybir.dt.float32
    f16 = mybir.dt.float16
    i16 = mybir.dt.int16
    AF = mybir.ActivationFunctionType
    AO = mybir.AluOpType

    xh_d = ins["xh"]          # [2C, XHF] f16 halo slice (+shifted copy)
    woffp_d = ins["woffph"]   # [128, 54] f16
    woffs_d = ins["woffsh"]   # [64, 54] f16
    wdcn_d = ins["wdcnh"]     # [128, 576] f16
    bdcn_d = ins["bdcn"]      # [64, 1] f32
    cadd_d = ins["cadd"]      # [128, 288] f32
    tab_d = ins["tab"].tensor  # [(TROWS+1)*128] f16 host-built gather table

    from concourse import library_config

    def ins_step0(apv, pos, num):
        # insert a step-0 (broadcast) dim at position `pos` of an AP view
        return bass.AP(
            tensor=apv.tensor,
            offset=apv.offset,
            ap=list(apv.ap[:pos]) + [[0, num]] + list(apv.ap[pos:]),
        )

    with ExitStack() as outer:
        tc = outer.enter_context(tile.TileContext(nc))
        nc.gpsimd.load_library(library_config.mlp)
        for _rep in range(repeat):
          with ExitStack() as ctx:
            consts = ctx.enter_context(tc.tile_pool(name="consts", bufs=1))
            sb = ctx.enter_context(tc.tile_pool(name="sb", bufs=1))
            pmain = ctx.enter_context(tc.tile_pool(name="pmain", bufs=10))
            vspool = ctx.enter_context(tc.tile_pool(name="vspool", bufs=4))
            stpool = ctx.enter_context(tc.tile_pool(name="stpool", bufs=4))
            setup_ctx = ctx.enter_context(ExitStack())

            # ---- constants in SBUF (f16 pre-built on host), spread across
            # queues so nothing delays the xh chunks or the gather chain:
            # conv weights on tensor (conv's own queue), cadd/bdcn on gpsimd
            # (before the idx folds), wdcnh on vector (needed latest).
            ident16 = consts.tile([128, 128], f16)
            make_identity(nc, ident16)
            woffph = consts.tile([2 * C, 3 * CH], f16)
            nc.scalar.dma_start(woffph[:], woffp_d[:])
            woffsh = consts.tile([C, 3 * CH], f16)
            nc.scalar.dma_start(woffsh[:], woffs_d[:])
            bdcn_sb = consts.tile([O, 1], f32)
            nc.scalar.dma_start(bdcn_sb[:], bdcn_d[:])
            wdcnh = consts.tile([128, K2 * O], f16)
            cadd_sb = consts.tile([128, ICH * CH], f32)

            # ---- load xh (host-built f16, shifted copy included) in small
            # chunks; early chunks split sync/scalar so the conv starts
            # early, the rest scalar-only to keep sync free for the
            # fold/replication chains that gate the gathers
            sbA = setup_ctx.enter_context(tc.tile_pool(name="sbA", bufs=1))
            xh = sbA.tile([128, XHF], f16)
            xcs = XHF // 16  # 616
            xengs = (nc.sync, nc.scalar)

            def emit_xh_chunks(ts, eng=None):
                for t in ts:
                    e = eng if eng is not None else xengs[t % 2]
                    e.dma_start(
                        xh[:, t * xcs : (t + 1) * xcs],
                        xh_d[:, t * xcs : (t + 1) * xcs],
                    )

            emit_xh_chunks([0])
            nc.sync.dma_start(cadd_sb[:], cadd_d[:])
            emit_xh_chunks(range(1, 6))
            # remaining chunks scalar-only, emitted now so their triggers
            # fire before the eviction copies clog the scalar queue
            emit_xh_chunks(range(6, 16), eng=nc.scalar)
            nc.scalar.dma_start(wdcnh[:], wdcn_d[:])

            if stage < 2:
                return
            xh3 = xh[:, :NPIX].rearrange("p (r s) -> p r s", s=RW)
            offs_sb = sb.tile([CH, Q], f16)

            # ---- prep tiles (sized for all 4 chunks; emitted per chunk)
            offsT = sb.tile([128, NI * CH], f32)
            pp = sb.tile([128, NI * CH], f32)
            fl = sb.tile([128, NI * CH], f32)
            gt = sb.tile([128, NI * CH], f32)
            wm1 = sb.tile([128, NI * CH], f32)
            idx16 = sb.tile([128, NI * K2], i16)
            wpp = sb.tile([128, NI * K2 * 4 * 2], f16)  # c-pair-duplicated weights
            idxw = sb.tile([128, K2 * (Q // 16)], i16)
            tmpw = sb.tile([16, 8 * K2 * NI], i16)
            pp3 = pp[:].rearrange("p (i c) -> p i c", c=CH)
            fl3 = fl[:].rearrange("p (i c) -> p i c", c=CH)
            fr = gt  # reuse after is_gt consumed
            fr3 = fr[:].rearrange("p (i c) -> p i c", c=CH)
            wm13 = wm1[:].rearrange("p (i c) -> p i c", c=CH)
            idx16i = idx16[:].rearrange("p (k i) -> p i k", i=NI)
            # wpp6: [p, k, i, b, a, c2]; the c2 pair holds the SAME value twice.
            # k-major so that after slicing k, (i, b, a) coalesce into one AP
            # dim (steps 8/4/2) and the weight operand fits in 3 free dims.
            wpp6 = wpp[:].rearrange(
                "p (k i b a c2) -> p k i b a c2", k=K2, i=NI, b=2, a=2
            )
            idxw3 = idxw[:].rearrange("p (k f) -> p k f", k=K2)
            idx163 = idx16[:].rearrange("p (k i) -> p k i", i=NI)
            idxw4 = idxw3[:, :, :].rearrange("p k (i j) -> p k i j", j=8)
            tmp3 = tmpw[:].rearrange("p (j k i) -> p j k i", j=8, k=K2)
            MAGIC = float(1 << 23)

            ps_conv = setup_ctx.enter_context(
                tc.tile_pool(name="ps_conv", bufs=3, space="PSUM")
            )
            ps_prep = setup_ctx.enter_context(
                tc.tile_pool(name="ps_prep", bufs=4, space="PSUM")
            )
            # per-chunk band source: indices are chunk-relative (see cadd)
            gsrcs = [
                bass.AP(
                    tensor=tab_d,
                    offset=u * ICH * RW * 128,
                    ap=[[128, 3040], [1, 256]],
                )
                for u in range(NCHUNK)
            ]
            v_tiles = {}

            def emit_conv_group(u):
                # offset conv for output rows [4u, 4u+4) -> offs_sb cols,
                # then the 4 rows' offset transposes into offsT
                psc = ps_conv.tile([CH, 512], f32, tag="conv")
                for kh in range(3):
                    rows = slice(u * 4 + kh + 3, u * 4 + kh + 7)
                    nc.tensor.matmul(
                        psc[:],
                        woffph[:, kh * CH : (kh + 1) * CH],
                        xh3[:, rows, 3:131],
                        start=(kh == 0),
                        stop=False,
                    )
                    nc.tensor.matmul(
                        psc[:],
                        woffsh[:, kh * CH : (kh + 1) * CH],
                        xh3[:C, rows, 5:133],
                        start=False,
                        stop=(kh == 2),
                    )
                nc.scalar.copy(offs_sb[:, u * 512 : (u + 1) * 512], psc[:])
                # chunk-0 offset-transpose evictions on the (idle) vector
                # engine so they aren't serialized behind conv evictions
                ev = nc.vector.tensor_copy if u < 4 else nc.scalar.copy
                for t in range(u * 4, u * 4 + 4):
                    pso = ps_prep.tile([128, CH], f16, tag="offT")
                    nc.tensor.transpose(
                        pso[:], offs_sb[:, t * 128 : (t + 1) * 128], ident16[:CH, :CH]
                    )
                    ev(offsT[:, t * CH : (t + 1) * CH], pso[:])

            def emit_prep(u):
                I = slice(u * ICH, (u + 1) * ICH)
                F = slice(u * ICH * CH, (u + 1) * ICH * CH)
                # critical-path ops (idx16) first; fr/wm1/wpp after
                nc.vector.tensor_tensor(pp[:, F], offsT[:, F], cadd_sb[:], AO.add)
                nc.vector.tensor_scalar_max(pp[:, F], pp[:, F], 0.0)
                yclu = min(ICH + 5.99, YCL - ICH * u)
                nc.vector.tensor_scalar_min(pp3[:, I, 0:9], pp3[:, I, 0:9], yclu)
                nc.vector.tensor_scalar_min(pp3[:, I, 9:18], pp3[:, I, 9:18], XCL)
                # exact floor for 0 <= x < 2^22: magic-add rounds to nearest
                # int, then subtract 1 where the rounded value exceeds x
                nc.vector.tensor_scalar(
                    fl[:, F], pp[:, F], MAGIC, MAGIC, AO.add, AO.subtract
                )
                nc.vector.tensor_tensor(gt[:, F], fl[:, F], pp[:, F], AO.is_gt)
                nc.vector.tensor_tensor(fl[:, F], fl[:, F], gt[:, F], AO.subtract)
                nc.vector.scalar_tensor_tensor(
                    idx16i[:, I, :], fl3[:, I, 0:9], 136.0, fl3[:, I, 9:18],
                    AO.mult, AO.add,
                )
                nc.vector.tensor_tensor(fr[:, F], pp[:, F], fl[:, F], AO.subtract)
                nc.vector.tensor_scalar(
                    wm1[:, F], fr[:, F], -1.0, 1.0, AO.mult, AO.add
                )
                # bilinear corner weight products, written twice (c2 pairs)
                for b in range(2):
                    wx = fr3[:, I, 9:18] if b else wm13[:, I, 9:18]
                    wxb = wx.rearrange("p i k -> p k i").broadcast_to(
                        [128, K2, ICH, 2]
                    )
                    for a in range(2):
                        wy = fr3[:, I, 0:9] if a else wm13[:, I, 0:9]
                        wyb = wy.rearrange("p i k -> p k i").broadcast_to(
                            [128, K2, ICH, 2]
                        )
                        nc.vector.tensor_tensor(
                            wpp6[:, :, I, b, a, :], wxb, wyb, AO.mult
                        )
                # wrap: partition fold 128->16 via bulk partition-shift DMAs
                # into (jj, k, i) staging, DVE free-dim permute to (k, i, jj),
                # then replication to 128 partitions by doubling DMAs.
                # all on sync: the scalar queue is owned by PSUM evictions
                # whose dependency stalls must not delay the idx chain
                for jj in range(8):
                    nc.sync.dma_start(
                        tmp3[:, jj, :, I], idx163[16 * jj : 16 * jj + 16, :, I]
                    )
                for k in range(K2):
                    nc.vector.tensor_copy(
                        idxw4[0:16, k, I, :],
                        tmp3[:, :, k, I].rearrange("p j i -> p i j"),
                    )
                FU = slice(u * (NIDX // 16), (u + 1) * (NIDX // 16))
                for g in (16, 32, 64):
                    nc.sync.dma_start(
                        idxw3[g : 2 * g, :, FU], idxw3[0:g, :, FU]
                    )
                if stage < 4:
                    return
                # gathers for all 9 taps of this chunk; the pmain ring
                # throttles Pool run-ahead automatically
                for k in range(K2):
                    v = pmain.tile([128, ICH * 256], f16, tag="V")
                    v_tiles[(u, k)] = v
                    v3 = v[:].rearrange("p (i e) -> p i e", e=256)
                    # two 1024-idx halves on different queues: smaller ring
                    # footprint avoids Pool head-of-line blocking on ring-full
                    for h in range(2):
                        nc.gpsimd.dma_gather(
                            v3[:, h * (ICH // 2) : (h + 1) * (ICH // 2), :],
                            gsrcs[u],
                            idxw3[
                                :,
                                k,
                                u * (NIDX // 16) + h * (NIDX // 32) : u * (NIDX // 16)
                                + (h + 1) * (NIDX // 32),
                            ],
                            num_idxs=NIDX // 2,
                            num_idxs_reg=NIDX // 2,
                            elem_size=256,
                            elem_step=128,
                            transpose=False,
                            single_packet=False,
                            queue_num=(u * K2 * 2 + k * 2 + h) % 4,
                        )

            # ---- offset conv, all groups upfront (PE runs them as xh
            # chunks arrive; each group's offset transposes follow it)
            for g in range(16):
                emit_conv_group(g)

            if stage < 5:
                return
            # ---- main loop: per chunk: prep (idx math + wrap + gathers),
            # then per tap: weight -> transpose -> deform matmul
            setup_ctx.close()
            ps_t = ctx.enter_context(tc.tile_pool(name="ps_t", bufs=2, space="PSUM"))
            ps_o = ctx.enter_context(tc.tile_pool(name="ps_o", bufs=1, space="PSUM"))
            obp = ctx.enter_context(tc.tile_pool(name="ob", bufs=2))
            # preps run three chunks ahead so the idx chain (DVE math ->
            # fold -> permute -> repl) never gates the gather stream
            emit_prep(0)
            emit_prep(1)
            emit_prep(2)
            for u in range(NCHUNK):
                if u + 3 < NCHUNK:
                    emit_prep(u + 3)
                I = slice(u * ICH, (u + 1) * ICH)
                psos = []
                for w in range(NIDX // 512):
                    pso_w = ps_o.tile([O, 512], f32, tag=f"out{w}", name=f"pso{w}")
                    psos.append(pso_w)
                for k in range(K2):
                    v = v_tiles[(u, k)]
                    # weighting multiply in the c-pair layout: all operands'
                    # innermost dim is [step 1, num 2] -> DVE 2x fp16 mode
                    v6 = v[:].rearrange(
                        "p (iba c32 c2) -> p iba c32 c2", iba=ICH * 4, c32=32
                    )
                    wq = ins_step0(
                        wpp6[:, k, I].rearrange("p i b a c -> p (i b a) c"), 2, 32
                    )
                    nc.vector.tensor_tensor(v6, v6, wq, AO.mult)
                    # sum the x-corner pair (b) -> [128, (i, a, c)]
                    v5 = v[:].rearrange("p (i b a c) -> p i b a c", i=ICH, b=2, a=2)
                    vs = vspool.tile([128, ICH * 128], f16, tag="VS")
                    vs3 = vs[:].rearrange("p (i e) -> p i e", e=128)
                    nc.vector.tensor_tensor(
                        vs3, v5[:, :, 0, :, :], v5[:, :, 1, :, :], AO.add
                    )
                    if stage < 6:
                        continue
                    stap = stpool.tile([128, ICH * 128], f16, tag="ST")
                    for h2 in range(2):
                        pt = ps_t.tile([128, (ICH // 2) * 128], f16, tag="T")
                        for i2 in range(ICH // 2):
                            i = h2 * (ICH // 2) + i2
                            nc.tensor.matmul(
                                pt[:, i2 * 128 : (i2 + 1) * 128],
                                vs3[:, i, :],
                                ident16,
                                is_transpose=True,
                                start=True,
                                stop=True,
                            )
                        nc.scalar.copy(
                            stap[
                                :, h2 * (ICH // 2) * 128 : (h2 + 1) * (ICH // 2) * 128
                            ],
                            pt[:],
                        )
                    for w in range(NIDX // 512):
                        nc.tensor.matmul(
                            psos[w][:],
                            wdcnh[:, k * O : (k + 1) * O],
                            stap[:, w * 512 : (w + 1) * 512],
                            start=(k == 0),
                            stop=(k == 8),
                        )
                if stage < 6:
                    continue
                for w in range(NIDX // 512):
                    ob = obp.tile([O, 512], f32, tag="ob")
                    if u == NCHUNK - 1:
                        nc.vector.tensor_scalar_add(ob[:], psos[w][:], bdcn_sb[:])
                    else:
                        nc.scalar.activation(
                            ob[:], psos[w][:], AF.Identity, bias=bdcn_sb[:]
                        )
                    nc.sync.dma_start(
                        out_ap[:, u * NIDX + w * 512 : u * NIDX + (w + 1) * 512], ob[:]
                    )


def _get_program():
    if "prog" in _cache:
        return _cache["prog"]
    import concourse.bacc as bacc
    import concourse.mybir as mybir

    f32 = mybir.dt.float32
    nc = bacc.Bacc(
        "TRN2",
        target_bir_lowering=False,
        debug=False,
        num_devices=8,
        num_swdge_queues=4,
    )
    f16 = mybir.dt.float16
    ins = {
        "xh": nc.dram_tensor("xh", [2 * C, XHF], f16, kind="ExternalInput").ap(),
        "woffph": nc.dram_tensor("woffph", [2 * C, 3 * CH], f16, kind="ExternalInput").ap(),
        "woffsh": nc.dram_tensor("woffsh", [C, 3 * CH], f16, kind="ExternalInput").ap(),
        "wdcnh": nc.dram_tensor("wdcnh", [2 * C, K2 * O], f16, kind="ExternalInput").ap(),
        "bdcn": nc.dram_tensor("bdcn", [O, 1], f32, kind="ExternalInput").ap(),
        "cadd": nc.dram_tensor("cadd", [128, ICH * CH], f32, kind="ExternalInput").ap(),
        "tab": nc.dram_tensor(
            "tab", [(TROWS + 1) * 128], f16, kind="ExternalInput"
        ).ap(),
    }
    out_ap = nc.dram_tensor("out", [O, Q], f32, kind="ExternalOutput").ap()
    build_tile_kernel(nc, ins, out_ap)
    nc.compile()
    _cache["prog"] = nc
    return nc


def make_in_maps(x, w_off, b_off, w_dcn, b_dcn):
    woffph, woffsh, wdcnh, bdcn = _prep_weights(
        np.asarray(w_off), np.asarray(b_off), np.asarray(w_dcn), np.asarray(b_dcn)
    )
    cadd = _build_consts(np.asarray(b_off))
    x = np.asarray(x)
    in_maps = []
    for m in range(8):
        b, h = m // 2, m % 2
        xi = np.zeros((C, RH, RW), dtype=np.float32)
        r0 = h * NI - HALO
        rlo, rhi = max(0, -r0), min(RH, H - r0)
        xi[:, rlo:rhi, HALO : HALO + W] = x[b, :, r0 + rlo : r0 + rhi, :]
        xif = xi.reshape(C, NPIX).astype(np.float16)
        # xh: rows 0-63 = xif (padded to XHF), rows 64-127 = xif shifted one
        # flat position left (the double-tap conv contraction reads it)
        xh = np.zeros((2 * C, XHF), dtype=np.float16)
        xh[:C, :NPIX] = xif
        xh[C:, : NPIX - 1] = xif.reshape(C, -1)[:, 1:]
        # host-built gather table: row r = [pixel r (64ch) | pixel r+136 (64ch)]
        xpadT = np.zeros((TROWS + 1 + RW, C), dtype=np.float16)
        xpadT[:NPIX] = xif.T
        tab = np.empty((TROWS + 1, 2 * C), dtype=np.float16)
        tab[:, :C] = xpadT[: TROWS + 1]
        tab[:, C:] = xpadT[RW : TROWS + 1 + RW]
        in_maps.append(
            {
                "xh": xh,
                "woffph": woffph,
                "woffsh": woffsh,
                "wdcnh": wdcnh,
                "bdcn": bdcn,
                "cadd": cadd,
                "tab": tab.reshape(-1),
            }
        )
    return in_maps


def kernel(x, w_off, b_off, w_dcn, b_dcn):
    from concourse import bass_utils

    nc = _get_program()
    in_maps = make_in_maps(x, w_off, b_off, w_dcn, b_dcn)
    res = bass_utils.run_bass_kernel_spmd(nc, in_maps, core_ids=list(range(8)))
    out = np.zeros((B, O, H, W), dtype=np.float32)
    for m in range(8):
        b, h = m // 2, m % 2
        out[b, :, h * NI : (h + 1) * NI, :] = res.results[m]["out"].reshape(O, NI, W)
    return out


# revision 47
# speedup vs baseline: 1.0543x; 1.0543x over previous
"""Deformable Conv2d (offset-conv -> bilinear sample -> 3x3 conv) on 8 NeuronCores.

Sharding: batch(4) x H-halves(2) -> 8 cores. Each core computes a [64, 64, 128]
slice of the output for one image. Inputs per core: a zero-padded halo slice of
its image plus (replicated) weights and index-offset constants.

Per-core device pipeline (v2):
  1. xh (f16, with the one-column-shifted copy on partitions 64-127) is built
     on the HOST and DMA'd straight into SBUF (no device cast / shift).
  2. offset conv emitted in 4 row-groups, interleaved with the 4 chunk preps
     so chunk 0's gathers start ~15us in instead of after the whole conv.
  3. prep(u): PE-transpose offsets to pixel-partitioned layout, DVE index
     math -> gather indices (int16) + bilinear corner weight products; the
     weight products are stored c-PAIR-DUPLICATED (innermost dim [2] step-1)
     so the weighting multiply qualifies for the DVE 2x fp16 perf mode
     (step-0 broadcast only one level up). Gathers for all 9 taps emitted
     here too - the v-tile pool ring throttles Pool run-ahead.
  4. dma_gather (Pool/SWDGE): one 512B descriptor per (tap, output pixel)
     fetches all 4 bilinear corners for all 64 channels.
  5. weight the gathered corners on DVE (2x-rate mult + pair add)
  6. PE transposes to put (y-corner, channel) on partitions
  7. deform conv: PE matmuls contracting (y-corner, channel) per tap,
     accumulating the 9 taps in PSUM; bias via ACT on eviction.
"""

import numpy as np
from contextlib import ExitStack

B, C, H, W, O = 4, 64, 128, 128, 64
K2, CH = 9, 18
NI = 64               # output rows per core
HALO = 4
RH, RW = 72, 136      # halo slice dims (rows [h*64-4, h*64+68), cols [-4, 132))
NPIX = RH * RW        # 9792
TCH = 77              # ceil(NPIX/128) transpose chunks for the gather table
XHF = TCH * 128       # 9856 padded pixel count
TROWS = XHF           # gather-table rows (one per padded pixel)
Q = NI * W            # 8192 output pixels per core
ICH = 16              # i-rows per main-loop chunk
NCHUNK = NI // ICH    # 4 chunks
NIDX = ICH * W        # 2048 gather indices per (tap, chunk)
YCL = 70.99
XCL = 134.99

_cache = {}


def _ch_perm(ch):
    # offset-conv output channel order: ch in [0,9) -> oy of tap ch,
    # ch in [9,18) -> ox of tap ch-9. Source channel in w_off layout:
    return 2 * ch if ch < 9 else 2 * (ch - 9) + 1


def _build_consts(b_off):
    """Host-side constant tensors (identical for every core). The offset-conv
    bias is folded in here (cadd is added to the raw conv output). The y base
    is CHUNK-RELATIVE (i%16), so cadd is 16-periodic in i and only one chunk's
    worth ([128, ICH*CH]) is stored: every chunk adds the same tile."""
    cadd = np.zeros((128, ICH, CH), dtype=np.float32)
    for chn in range(CH):
        if chn < 9:
            kh = chn // 3
            cadd[:, :, chn] = (np.arange(ICH, dtype=np.float32) + 3 + kh)[None, :]
        else:
            kw = (chn - 9) % 3
            cadd[:, :, chn] = (np.arange(128, dtype=np.float32) + 3 + kw)[:, None]
        cadd[:, :, chn] += b_off[_ch_perm(chn)]
    return cadd.reshape(128, ICH * CH)


def _prep_weights(w_off, b_off, w_dcn, b_dcn):
    # Offset-conv lhsT, packed for double-tap contraction: for each kernel row
    # kh, taps (kh,0) and (kh,1) contract together over K=128 (the image copy
    # on partitions 64-127 is pre-shifted one column), tap (kh,2) is a K=64
    # single. woffp[kh]: [128, 18]; woffs[kh]: [64, 18].
    woffp = np.zeros((3, 2 * C, CH), dtype=np.float32)
    woffs = np.zeros((3, C, CH), dtype=np.float32)
    for kh in range(3):
        for chn in range(CH):
            woffp[kh, :C, chn] = w_off[_ch_perm(chn), :, kh, 0]
            woffp[kh, C:, chn] = w_off[_ch_perm(chn), :, kh, 1]
            woffs[kh, :, chn] = w_off[_ch_perm(chn), :, kh, 2]
    # wdcn_r[k, a*64+c, o] : lhsT for deform conv tap k, replicated over the
    # y-corner index a (the transposed sampled tensor has (a, c) on partitions)
    wdcn_r = np.zeros((K2, 2 * C, O), dtype=np.float32)
    for k in range(K2):
        kh, kw = k // 3, k % 3
        wdcn_r[k, :C, :] = w_dcn[:, :, kh, kw].T
        wdcn_r[k, C:, :] = w_dcn[:, :, kh, kw].T
    # device-layout f16 weights (partition-major, tap-interleaved free dim)
    woffph = np.ascontiguousarray(
        woffp.transpose(1, 0, 2).reshape(2 * C, 3 * CH)
    ).astype(np.float16)
    woffsh = np.ascontiguousarray(
        woffs.transpose(1, 0, 2).reshape(C, 3 * CH)
    ).astype(np.float16)
    wdcnh = np.ascontiguousarray(
        wdcn_r.transpose(1, 0, 2).reshape(2 * C, K2 * O)
    ).astype(np.float16)
    return woffph, woffsh, wdcnh, b_dcn.reshape(O, 1).astype(np.float32)


def build_tile_kernel(nc, ins, out_ap, stage=99, repeat=1):
    """Emit the per-core program. ins: dict name -> AP (DRAM).
    stage truncates the pipeline for debugging (99 = full)."""
    import concourse.bass as bass
    import concourse.mybir as mybir
    import concourse.tile as tile
    from concourse.masks import make_identity

    f32 = mybir.dt.float32
    f16 = mybir.dt.float16
    i16 = mybir.dt.int16
    AF = mybir.ActivationFunctionType
    AO = mybir.AluOpType

    xh_d = ins["xh"]          # [2C, XHF] f16 halo slice (+shifted copy)
    woffp_d = ins["woffph"]   # [128, 54] f16
    woffs_d = ins["woffsh"]   # [64, 54] f16
    wdcn_d = ins["wdcnh"]     # [128, 576] f16
    bdcn_d = ins["bdcn"]      # [64, 1] f32
    cadd_d = ins["cadd"]      # [128, 288] f32
    tab_d = ins["tab"].tensor  # [(TROWS+1)*128] f16 host-built gather table

    from concourse import library_config

    def ins_step0(apv, pos, num):
        # insert a step-0 (broadcast) dim at position `pos` of an AP view
        return bass.AP(
            tensor=apv.tensor,
            offset=apv.offset,
            ap=list(apv.ap[:pos]) + [[0, num]] + list(apv.ap[pos:]),
        )

    with ExitStack() as outer:
        tc = outer.enter_context(tile.TileContext(nc))
        nc.gpsimd.load_library(library_config.mlp)
        for _rep in range(repeat):
          with ExitStack() as ctx:
            consts = ctx.enter_context(tc.tile_pool(name="consts", bufs=1))
            sb = ctx.enter_context(tc.tile_pool(name="sb", bufs=1))
            pmain = ctx.enter_context(tc.tile_pool(name="pmain", bufs=10))
            vspool = ctx.enter_context(tc.tile_pool(name="vspool", bufs=4))
            stpool = ctx.enter_context(tc.tile_pool(name="stpool", bufs=4))
            setup_ctx = ctx.enter_context(ExitStack())

            # ---- constants in SBUF (f16 pre-built on host), spread across
            # queues so nothing delays the xh chunks or the gather chain:
            # conv weights on tensor (conv's own queue), cadd/bdcn on gpsimd
            # (before the idx folds), wdcnh on vector (needed latest).
            ident16 = consts.tile([128, 128], f16)
            make_identity(nc, ident16)
            woffph = consts.tile([2 * C, 3 * CH], f16)
            nc.scalar.dma_start(woffph[:], woffp_d[:])
            woffsh = consts.tile([C, 3 * CH], f16)
            nc.scalar.dma_start(woffsh[:], woffs_d[:])
            bdcn_sb = consts.tile([O, 1], f32)
            nc.scalar.dma_start(bdcn_sb[:], bdcn_d[:])
            wdcnh = consts.tile([128, K2 * O], f16)
            cadd_sb = consts.tile([128, ICH * CH], f32)

            # ---- load xh (host-built f16, shifted copy included) in small
            # chunks; early chunks split sync/scalar so the conv starts
            # early, the rest scalar-only to keep sync free for the
            # fold/replication chains that gate the gathers
            sbA = setup_ctx.enter_context(tc.tile_pool(name="sbA", bufs=1))
            xh = sbA.tile([128, XHF], f16)
            xcs = XHF // 16  # 616
            xengs = (nc.sync, nc.scalar)

            def emit_xh_chunks(ts, eng=None):
                for t in ts:
                    e = eng if eng is not None else xengs[t % 2]
                    e.dma_start(
                        xh[:, t * xcs : (t + 1) * xcs],
                        xh_d[:, t * xcs : (t + 1) * xcs],
                    )

            emit_xh_chunks([0])
            nc.sync.dma_start(cadd_sb[:], cadd_d[:])
            emit_xh_chunks(range(1, 6))
            # remaining chunks scalar-only, emitted now so their triggers
            # fire before the eviction copies clog the scalar queue
            emit_xh_chunks(range(6, 16), eng=nc.scalar)
            nc.scalar.dma_start(wdcnh[:], wdcn_d[:])

            if stage < 2:
                return
            xh3 = xh[:, :NPIX].rearrange("p (r s) -> p r s", s=RW)
            offs_sb = sb.tile([CH, Q], f16)

            # ---- prep tiles (sized for all 4 chunks; emitted per chunk)
            offsT = sb.tile([128, NI * CH], f32)
            pp = sb.tile([128, NI * CH], f32)
            fl = sb.tile([128, NI * CH], f32)
            gt = sb.tile([128, NI * CH], f32)
            wm1 = sb.tile([128, NI * CH], f32)
            idx16 = sb.tile([128, NI * K2], i16)
            wpp = sb.tile([128, NI * K2 * 4 * 2], f16)  # c-pair-duplicated weights
            idxw = sb.tile([128, K2 * (Q // 16)], i16)
            tmpw = sb.tile([16, 8 * K2 * NI], i16)
            pp3 = pp[:].rearrange("p (i c) -> p i c", c=CH)
            fl3 = fl[:].rearrange("p (i c) -> p i c", c=CH)
            fr = gt  # reuse after is_gt consumed
            fr3 = fr[:].rearrange("p (i c) -> p i c", c=CH)
            wm13 = wm1[:].rearrange("p (i c) -> p i c", c=CH)
            idx16i = idx16[:].rearrange("p (k i) -> p i k", i=NI)
            # wpp6: [p, k, i, b, a, c2]; the c2 pair holds the SAME value twice.
            # k-major so that after slicing k, (i, b, a) coalesce into one AP
            # dim (steps 8/4/2) and the weight operand fits in 3 free dims.
            wpp6 = wpp[:].rearrange(
                "p (k i b a c2) -> p k i b a c2", k=K2, i=NI, b=2, a=2
            )
            idxw3 = idxw[:].rearrange("p (k f) -> p k f", k=K2)
            idx163 = idx16[:].rearrange("p (k i) -> p k i", i=NI)
            idxw4 = idxw3[:, :, :].rearrange("p k (i j) -> p k i j", j=8)
            tmp3 = tmpw[:].rearrange("p (j k i) -> p j k i", j=8, k=K2)
            MAGIC = float(1 << 23)

            ps_conv = setup_ctx.enter_context(
                tc.tile_pool(name="ps_conv", bufs=3, space="PSUM")
            )
            ps_prep = setup_ctx.enter_context(
                tc.tile_pool(name="ps_prep", bufs=4, space="PSUM")
            )
            # per-chunk band source: indices are chunk-relative (see cadd)
            gsrcs = [
                bass.AP(
                    tensor=tab_d,
                    offset=u * ICH * RW * 128,
                    ap=[[128, 3040], [1, 256]],
                )
                for u in range(NCHUNK)
            ]
            v_tiles = {}

            def emit_conv_group(u):
                # offset conv for output rows [4u, 4u+4) -> offs_sb cols,
                # then the 4 rows' offset transposes into offsT
                psc = ps_conv.tile([CH, 512], f32, tag="conv")
                for kh in range(3):
                    rows = slice(u * 4 + kh + 3, u * 4 + kh + 7)
                    nc.tensor.matmul(
                        psc[:],
                        woffph[:, kh * CH : (kh + 1) * CH],
                        xh3[:, rows, 3:131],
                        start=(kh == 0),
                        stop=False,
                    )
                    nc.tensor.matmul(
                        psc[:],
                        woffsh[:, kh * CH : (kh + 1) * CH],
                        xh3[:C, rows, 5:133],
                        start=False,
                        stop=(kh == 2),
                    )
                nc.scalar.copy(offs_sb[:, u * 512 : (u + 1) * 512], psc[:])
                # chunk-0 offset-transpose evictions on the (idle) vector
                # engine so they aren't serialized behind conv evictions
                ev = nc.vector.tensor_copy if u < 4 else nc.scalar.copy
                for t in range(u * 4, u * 4 + 4):
                    pso = ps_prep.tile([128, CH], f16, tag="offT")
                    nc.tensor.transpose(
                        pso[:], offs_sb[:, t * 128 : (t + 1) * 128], ident16[:CH, :CH]
                    )
                    ev(offsT[:, t * CH : (t + 1) * CH], pso[:])

            def emit_prep(u):
                I = slice(u * ICH, (u + 1) * ICH)
                F = slice(u * ICH * CH, (u + 1) * ICH * CH)
                # critical-path ops (idx16) first; fr/wm1/wpp after
                nc.vector.tensor_tensor(pp[:, F], offsT[:, F], cadd_sb[:], AO.add)
                nc.vector.tensor_scalar_max(pp[:, F], pp[:, F], 0.0)
                yclu = min(ICH + 5.99, YCL - ICH * u)
                nc.vector.tensor_scalar_min(pp3[:, I, 0:9], pp3[:, I, 0:9], yclu)
                nc.vector.tensor_scalar_min(pp3[:, I, 9:18], pp3[:, I, 9:18], XCL)
                # exact floor for 0 <= x < 2^22: magic-add rounds to nearest
                # int, then subtract 1 where the rounded value exceeds x
                nc.vector.tensor_scalar(
                    fl[:, F], pp[:, F], MAGIC, MAGIC, AO.add, AO.subtract
                )
                nc.vector.tensor_tensor(gt[:, F], fl[:, F], pp[:, F], AO.is_gt)
                nc.vector.tensor_tensor(fl[:, F], fl[:, F], gt[:, F], AO.subtract)
                nc.vector.scalar_tensor_tensor(
                    idx16i[:, I, :], fl3[:, I, 0:9], 136.0, fl3[:, I, 9:18],
                    AO.mult, AO.add,
                )
                nc.vector.tensor_tensor(fr[:, F], pp[:, F], fl[:, F], AO.subtract)
                nc.vector.tensor_scalar(
                    wm1[:, F], fr[:, F], -1.0, 1.0, AO.mult, AO.add
                )
                # bilinear corner weight products, written twice (c2 pairs)
                for b in range(2):
                    wx = fr3[:, I, 9:18] if b else wm13[:, I, 9:18]
                    wxb = wx.rearrange("p i k -> p k i").broadcast_to(
                        [128, K2, ICH, 2]
                    )
                    for a in range(2):
                        wy = fr3[:, I, 0:9] if a else wm13[:, I, 0:9]
                        wyb = wy.rearrange("p i k -> p k i").broadcast_to(
                            [128, K2, ICH, 2]
                        )
                        nc.vector.tensor_tensor(
                            wpp6[:, :, I, b, a, :], wxb, wyb, AO.mult
                        )
                # wrap: partition fold 128->16 via bulk partition-shift DMAs
                # into (jj, k, i) staging, DVE free-dim permute to (k, i, jj),
                # then replication to 128 partitions by doubling DMAs.
                # all on sync: the scalar queue is owned by PSUM evictions
                # whose dependency stalls must not delay the idx chain
                for jj in range(8):
                    nc.sync.dma_start(
                        tmp3[:, jj, :, I], idx163[16 * jj : 16 * jj + 16, :, I]
                    )
                for k in range(K2):
                    nc.vector.tensor_copy(
                        idxw4[0:16, k, I, :],
                        tmp3[:, :, k, I].rearrange("p j i -> p i j"),
                    )
                FU = slice(u * (NIDX // 16), (u + 1) * (NIDX // 16))
                for g in (16, 32, 64):
                    nc.sync.dma_start(
                        idxw3[g : 2 * g, :, FU], idxw3[0:g, :, FU]
                    )
                if stage < 4:
                    return
                # gathers for all 9 taps of this chunk; the pmain ring
                # throttles Pool run-ahead automatically
                for k in range(K2):
                    v = pmain.tile([128, ICH * 256], f16, tag="V")
                    v_tiles[(u, k)] = v
                    v3 = v[:].rearrange("p (i e) -> p i e", e=256)
                    # two 1024-idx halves on different queues: smaller ring
                    # footprint avoids Pool head-of-line blocking on ring-full
                    for h in range(2):
                        nc.gpsimd.dma_gather(
                            v3[:, h * (ICH // 2) : (h + 1) * (ICH // 2), :],
                            gsrcs[u],
                            idxw3[
                                :,
                                k,
                                u * (NIDX // 16) + h * (NIDX // 32) : u * (NIDX // 16)
                                + (h + 1) * (NIDX // 32),
                            ],
                            num_idxs=NIDX // 2,
                            num_idxs_reg=NIDX // 2,
                            elem_size=256,
                            elem_step=128,
                            transpose=False,
                            single_packet=False,
                            queue_num=(u * K2 * 2 + k * 2 + h) % 4,
                        )

            # ---- offset conv, all groups upfront (PE runs them as xh
            # chunks arrive; each group's offset transposes follow it)
            for g in range(16):
                emit_conv_group(g)

            if stage < 5:
                return
            # ---- main loop: per chunk: prep (idx math + wrap + gathers),
            # then per tap: weight -> transpose -> deform matmul
            setup_ctx.close()
            ps_t = ctx.enter_context(tc.tile_pool(name="ps_t", bufs=2, space="PSUM"))
            ps_o = ctx.enter_context(tc.tile_pool(name="ps_o", bufs=1, space="PSUM"))
            obp = ctx.enter_context(tc.tile_pool(name="ob", bufs=2))
            # preps run three chunks ahead so the idx chain (DVE math ->
            # fold -> permute -> repl) never gates the gather stream
            emit_prep(0)
            emit_prep(1)
            for u in range(NCHUNK):
                if u + 2 < NCHUNK:
                    emit_prep(u + 2)
                I = slice(u * ICH, (u + 1) * ICH)
                psos = []
                for w in range(NIDX // 512):
                    pso_w = ps_o.tile([O, 512], f32, tag=f"out{w}", name=f"pso{w}")
                    psos.append(pso_w)
                for k in range(K2):
                    v = v_tiles[(u, k)]
                    # weighting multiply in the c-pair layout: all operands'
                    # innermost dim is [step 1, num 2] -> DVE 2x fp16 mode
                    v6 = v[:].rearrange(
                        "p (iba c32 c2) -> p iba c32 c2", iba=ICH * 4, c32=32
                    )
                    wq = ins_step0(
                        wpp6[:, k, I].rearrange("p i b a c -> p (i b a) c"), 2, 32
                    )
                    nc.vector.tensor_tensor(v6, v6, wq, AO.mult)
                    # sum the x-corner pair (b) -> [128, (i, a, c)]
                    v5 = v[:].rearrange("p (i b a c) -> p i b a c", i=ICH, b=2, a=2)
                    vs = vspool.tile([128, ICH * 128], f16, tag="VS")
                    vs3 = vs[:].rearrange("p (i e) -> p i e", e=128)
                    nc.vector.tensor_tensor(
                        vs3, v5[:, :, 0, :, :], v5[:, :, 1, :, :], AO.add
                    )
                    if stage < 6:
                        continue
                    stap = stpool.tile([128, ICH * 128], f16, tag="ST")
                    for h2 in range(2):
                        pt = ps_t.tile([128, (ICH // 2) * 128], f16, tag="T")
                        for i2 in range(ICH // 2):
                            i = h2 * (ICH // 2) + i2
                            nc.tensor.matmul(
                                pt[:, i2 * 128 : (i2 + 1) * 128],
                                vs3[:, i, :],
                                ident16,
                                is_transpose=True,
                                start=True,
                                stop=True,
                            )
                        nc.scalar.copy(
                            stap[
                                :, h2 * (ICH // 2) * 128 : (h2 + 1) * (ICH // 2) * 128
                            ],
                            pt[:],
                        )
                    for w in range(NIDX // 512):
                        nc.tensor.matmul(
                            psos[w][:],
                            wdcnh[:, k * O : (k + 1) * O],
                            stap[:, w * 512 : (w + 1) * 512],
                            start=(k == 0),
                            stop=(k == 8),
                        )
                if stage < 6:
                    continue
                for w in range(NIDX // 512):
                    ob = obp.tile([O, 512], f32, tag="ob")
                    if u == NCHUNK - 1:
                        nc.vector.tensor_scalar_add(ob[:], psos[w][:], bdcn_sb[:])
                    else:
                        nc.scalar.activation(
                            ob[:], psos[w][:], AF.Identity, bias=bdcn_sb[:]
                        )
                    nc.sync.dma_start(
                        out_ap[:, u * NIDX + w * 512 : u * NIDX + (w + 1) * 512], ob[:]
                    )


def _get_program():
    if "prog" in _cache:
        return _cache["prog"]
    import concourse.bacc as bacc
    import concourse.mybir as mybir

    f32 = mybir.dt.float32
    nc = bacc.Bacc(
        "TRN2",
        target_bir_lowering=False,
        debug=False,
        num_devices=8,
        num_swdge_queues=4,
    )
    f16 = mybir.dt.float16
    ins = {
        "xh": nc.dram_tensor("xh", [2 * C, XHF], f16, kind="ExternalInput").ap(),
        "woffph": nc.dram_tensor("woffph", [2 * C, 3 * CH], f16, kind="ExternalInput").ap(),
        "woffsh": nc.dram_tensor("woffsh", [C, 3 * CH], f16, kind="ExternalInput").ap(),
        "wdcnh": nc.dram_tensor("wdcnh", [2 * C, K2 * O], f16, kind="ExternalInput").ap(),
        "bdcn": nc.dram_tensor("bdcn", [O, 1], f32, kind="ExternalInput").ap(),
        "cadd": nc.dram_tensor("cadd", [128, ICH * CH], f32, kind="ExternalInput").ap(),
        "tab": nc.dram_tensor(
            "tab", [(TROWS + 1) * 128], f16, kind="ExternalInput"
        ).ap(),
    }
    out_ap = nc.dram_tensor("out", [O, Q], f32, kind="ExternalOutput").ap()
    build_tile_kernel(nc, ins, out_ap)
    nc.compile()
    _cache["prog"] = nc
    return nc


def make_in_maps(x, w_off, b_off, w_dcn, b_dcn):
    woffph, woffsh, wdcnh, bdcn = _prep_weights(
        np.asarray(w_off), np.asarray(b_off), np.asarray(w_dcn), np.asarray(b_dcn)
    )
    cadd = _build_consts(np.asarray(b_off))
    x = np.asarray(x)
    in_maps = []
    for m in range(8):
        b, h = m // 2, m % 2
        xi = np.zeros((C, RH, RW), dtype=np.float32)
        r0 = h * NI - HALO
        rlo, rhi = max(0, -r0), min(RH, H - r0)
        xi[:, rlo:rhi, HALO : HALO + W] = x[b, :, r0 + rlo : r0 + rhi, :]
        xif = xi.reshape(C, NPIX).astype(np.float16)
        # xh: rows 0-63 = xif (padded to XHF), rows 64-127 = xif shifted one
        # flat position left (the double-tap conv contraction reads it)
        xh = np.zeros((2 * C, XHF), dtype=np.float16)
        xh[:C, :NPIX] = xif
        xh[C:, : NPIX - 1] = xif.reshape(C, -1)[:, 1:]
        # host-built gather table: row r = [pixel r (64ch) | pixel r+136 (64ch)]
        xpadT = np.zeros((TROWS + 1 + RW, C), dtype=np.float16)
        xpadT[:NPIX] = xif.T
        tab = np.empty((TROWS + 1, 2 * C), dtype=np.float16)
        tab[:, :C] = xpadT[: TROWS + 1]
        tab[:, C:] = xpadT[RW : TROWS + 1 + RW]
        in_maps.append(
            {
                "xh": xh,
                "woffph": woffph,
                "woffsh": woffsh,
                "wdcnh": wdcnh,
                "bdcn": bdcn,
                "cadd": cadd,
                "tab": tab.reshape(-1),
            }
        )
    return in_maps


def kernel(x, w_off, b_off, w_dcn, b_dcn):
    from concourse import bass_utils

    nc = _get_program()
    in_maps = make_in_maps(x, w_off, b_off, w_dcn, b_dcn)
    res = bass_utils.run_bass_kernel_spmd(nc, in_maps, core_ids=list(range(8)))
    out = np.zeros((B, O, H, W), dtype=np.float32)
    for m in range(8):
        b, h = m // 2, m % 2
        out[b, :, h * NI : (h + 1) * NI, :] = res.results[m]["out"].reshape(O, NI, W)
    return out
